# revision 42
# baseline (speedup 1.0000x reference)
"""Trainium2 Bass kernel for nn_Conv: per-token 16x8 image, 3x3 valid conv,
output flattened to first 84 of 128 slots, rest zero, ReLU.

Strategy (hardcoded for x:[256,1024,128] fp32, kernel:[3,3] fp32, 8 cores):
  - Pure data parallel: batch 256 -> 32 per core, 32768 tokens per core.
  - conv == x[tok, 128] @ M[128, 84] with M built on host from the 3x3 kernel.
  - Everything in bf16 (correctness gate is 2e-2; bf16 conv lands ~1e-3):
    halves both directions of HBM traffic vs fp32.
  - Host pre-transposes x to pixel-major xT[128, 32768] bf16 per core, so the
    device needs NO PE transpose: matmul(lhsT=M[128px,84], rhs=xT[:, n0:n1])
    -> PSUM [84, 512] fp32, one bank per matmul, 8 banks cycling.
  - ReLU + fp32->bf16 cast fused into PSUM evacuation, alternating DVE/ACT.
  - Device writes only the 84 live output rows, transposed [84, 32768] bf16;
    host transposes back and pads the 44 zero columns. Device traffic per
    core: 8.4 MB in + 5.5 MB out (vs 16.8 + 16.8 for the fp32 kernel).
  - 8 input DMAs (~1 MB each) on the 8 HWDGE lanes, 8 output DMAs on the 8
    SWDGE lanes; M rides in front of chunk 0's tile instead of a 17th DMA.
  - Walrus allows one sync-wait per instruction: _split_excess_waits moves
    extras onto same-engine NoOps.
"""

from contextlib import ExitStack

import ml_dtypes
import numpy as np

import concourse.bass as bass
import concourse.tile as tile
from concourse import mybir
from concourse.bass_utils import run_bass_kernel_spmd

L, W, K = 16, 8, 3
B, S = 256, 1024
PX = L * W  # 128 pixels per token
OUT = (L - K + 1) * (W - K + 1)  # 84 conv outputs per token
N_CORES = 8
B_SHARD = B // N_CORES  # 32
TOKENS = B_SHARD * S  # 32768 tokens per core

BLK = 128  # tokens per matmul (stationary lhsT = xT block [128 px, 128 tok])
P = 128
# Input chunk sizes: big chunks amortize DMA fixed cost; the last two are
# half-size so the pipeline drain tail is shorter.
CHUNK_SIZES = [4096] * 6 + [2048] * 2 + [1024] * 4
assert sum(CHUNK_SIZES) == TOKENS
CHUNK0 = CHUNK_SIZES[0]
DEFER_K = 3  # out-DMA for chunk c waits for the input of chunk c+DEFER_K


def _bank_split(nblocks: int):
    """Split a chunk's 128-token blocks into PSUM tiles of <=6 blocks
    (6 * 84 = 504 fp32 columns fits one 512-column PSUM bank)."""
    out = []
    while nblocks > 0:
        take = min(6, nblocks)
        out.append(take)
        nblocks -= take
    return out

BF16 = ml_dtypes.bfloat16


def _build_conv_matrix(kernel3x3: np.ndarray) -> np.ndarray:
    """M[p, o]: coefficient of pixel p in conv output slot o."""
    m = np.zeros((PX, OUT), dtype=np.float32)
    oh, ow = L - K + 1, W - K + 1
    for oy in range(oh):
        for ox in range(ow):
            for ky in range(K):
                for kx in range(K):
                    m[(oy + ky) * W + (ox + kx), oy * ow + ox] += kernel3x3[ky, kx]
    return m


def _build_program():
    nc = bass.Bass(
        "TRN2", target_bir_lowering=False, debug=False, num_devices=N_CORES
    )
    f32 = mybir.dt.float32
    bf16 = mybir.dt.bfloat16
    # chunk 0 input: M[128, 84] columns, then the first CHUNK0 token columns
    x0m_ap = nc.dram_tensor("x0m", [P, OUT + CHUNK0], bf16, kind="ExternalInput").ap()
    xr_ap = nc.dram_tensor(
        "xr", [P, TOKENS - CHUNK0], bf16, kind="ExternalInput"
    ).ap()
    # Output is token-block-major: row p, col b*84+o = conv slot o of token
    # b*128+p.  All 128 partitions carry useful bytes, so the out-DMA spans
    # all 16 SBUF AXI ports and moves only the 84 live slots per token.
    # Each chunk's slab carries one extra leading "dependency" column (host
    # strips it): the out-DMA is deferred on an input DEFER_K chunks ahead,
    # which keeps a backlog of ready output so the port fabric stays
    # saturated through the pipeline drain instead of dribbling.
    out_ap = nc.dram_tensor(
        "out",
        [P, (TOKENS // BLK) * OUT + len(CHUNK_SIZES)],
        bf16,
        kind="ExternalOutput",
    ).ap()

    with tile.TileContext(nc) as tc, ExitStack() as ctx:
        consts = ctx.enter_context(tc.tile_pool(name="consts", bufs=1))
        x_pool = ctx.enter_context(tc.tile_pool(name="x", bufs=6))
        xt_pool = ctx.enter_context(tc.tile_pool(name="xt", bufs=4))
        o_pool = ctx.enter_context(tc.tile_pool(name="o", bufs=len(CHUNK_SIZES)))
        ps_pool = ctx.enter_context(tc.tile_pool(name="ps", bufs=6, space="PSUM"))
        wps_pool = ctx.enter_context(tc.tile_pool(name="wps", bufs=2, space="PSUM"))

        # Chunk 0 + M, persistent (M is the moving operand of every matmul).
        # Split across both HWDGE rings so the stream ramps at full rate
        # from the first byte (rings are FIFO per engine, so one ring would
        # serialize the two pieces).
        x0m_tile = consts.tile([P, OUT + CHUNK0], bf16)
        half = (OUT + CHUNK0) // 2
        nc.sync.dma_start(x0m_tile[:, :half], x0m_ap[:, :half])
        nc.scalar.dma_start(x0m_tile[:, half:], x0m_ap[:, half:])
        m_sb = x0m_tile[:, :OUT]

        # PE pre-warm + per-chunk filler: the HAM clock gate needs ~3.4us of
        # sustained matmul activity to lift PE from 1.2 to 2.4 GHz, and
        # re-throttles after idle gaps of ~1.7us+.  Dummy matmuls on a
        # zeroed tile keep PE busy through the DMA fill and between chunks
        # so every real matmul runs at full clock.
        warm = consts.tile([P, 512 + P], bf16)
        nc.gpsimd.memset(warm[:], 0.0)
        wcount = [0]

        def dummy_mms(n):
            for _ in range(n):
                w = wcount[0]
                wcount[0] += 1
                wps = wps_pool.tile([P, 512], f32, name=f"warm{w}", tag="wps")
                nc.tensor.matmul(
                    wps[:],
                    lhsT=warm[:, 512 : 512 + P],
                    rhs=warm[:, :512],
                    start=True,
                    stop=True,
                )

        dummy_mms(16)

        ev = 0  # evacuation op counter (alternates DVE/ACT)
        N = len(CHUNK_SIZES)
        TAIL = N - 4
        tail_tiles = {}
        x_tiles = {}
        o_tiles = {}
        starts = [sum(CHUNK_SIZES[:i]) for i in range(N)]

        def emit_out(c, dep_tile):
            """Emit chunk c's dependency-column copy + out-DMA."""
            nblocks = CHUNK_SIZES[c] // BLK
            o_t = o_tiles[c]
            nc.gpsimd.tensor_copy(o_t[:, 0:1], dep_tile[:, 0:1])
            col0 = (starts[c] // BLK) * OUT + c  # slab offset incl. dep cols
            out_eng = nc.gpsimd if c < TAIL else nc.sync
            out_eng.dma_start(
                out_ap[:, col0 : col0 + 1 + nblocks * OUT], o_t[:]
            )

        tok0 = 0  # starting token of current chunk
        for c, csize in enumerate(CHUNK_SIZES):
            nblocks = csize // BLK
            if c == 0:
                x_tile, off = x0m_tile, OUT
            elif c >= TAIL:
                x_tile, off = tail_tiles[c], 0
            else:
                x_tile = x_pool.tile([P, csize], bf16, name=f"x{c}", tag="x")
                off = 0
                # Alternate the two HWDGE rings (sync / scalar) so
                # descriptor generation is not serialized on one engine.
                dma_eng = nc.sync if c % 2 == 0 else nc.scalar
                dma_eng.dma_start(
                    x_tile[:], xr_ap[:, tok0 - CHUNK0 : tok0 - CHUNK0 + csize]
                )
            x_tiles[c] = x_tile
            if c == TAIL - 1:
                # Hoist the drain-tail in-DMAs here: their out-DMAs ride the
                # sync HWDGE ring, which is FIFO per engine, so the ins must
                # precede the outs in program order to avoid head-of-line
                # blocking.  Execution is still gated by buffer-release sems.
                for ct in range(TAIL, N):
                    xt_t = xt_pool.tile(
                        [P, CHUNK_SIZES[ct]], bf16, name=f"x{ct}", tag="xt"
                    )
                    tail_tiles[ct] = xt_t
                    x_tiles[ct] = xt_t
                    s0 = starts[ct] - CHUNK0
                    dma_eng = nc.sync if ct % 2 == 0 else nc.scalar
                    dma_eng.dma_start(
                        xt_t[:], xr_ap[:, s0 : s0 + CHUNK_SIZES[ct]]
                    )
            o_tile = o_pool.tile([P, 1 + nblocks * OUT], bf16, name=f"o{c}", tag="o")
            o_tiles[c] = o_tile

            b = 0  # block index within chunk
            for nblk in _bank_split(nblocks):
                ps = ps_pool.tile(
                    [P, nblk * OUT], f32, name=f"ps{c}_{b}", tag="ps"
                )
                for k in range(nblk):
                    t0 = (b + k) * BLK
                    nc.tensor.matmul(
                        ps[:, k * OUT : (k + 1) * OUT],
                        lhsT=x_tile[:, off + t0 : off + t0 + BLK],
                        rhs=m_sb,
                        start=True,
                        stop=True,
                    )
                osl = o_tile[:, 1 + b * OUT : 1 + (b + nblk) * OUT]
                if ev % 2 == 0:
                    nc.vector.tensor_scalar_max(osl, ps[:], 0.0)
                else:
                    nc.scalar.activation(
                        osl, ps[:], mybir.ActivationFunctionType.Relu
                    )
                ev += 1
                b += nblk
            # ~0.2us of filler per 3 blocks keeps inter-chunk PE gaps under
            # the ~1.7us clock-gate re-throttle threshold.  None in the
            # drain tail: the PE queue is in-order, so filler there delays
            # the final real matmuls directly.
            if c < TAIL:
                dummy_mms(nblocks // 3)

            # Deferred outputs: chunk c - DEFER_K's out-DMA is emitted here,
            # with a dependency column read from this chunk's input tile.
            if c >= DEFER_K:
                emit_out(c - DEFER_K, x_tile)
            tok0 += csize

        # Flush the last DEFER_K chunks, deferred on the final input.
        for c in range(N - DEFER_K, N):
            emit_out(c, x_tiles[N - 1])

    _split_excess_waits(nc)
    return nc


_SKIP_TYPES = ("Branch", "SemWait")


def _split_excess_waits(nc):
    """Move all but one sync wait onto injected same-engine NoOps.

    Walrus allows a single sync-wait slot per compute/DMA instruction, but
    the tile scheduler can emit several (data deps + its event-accel /
    bank-safety pacing waits).  A NoOp on the same engine immediately before
    the instruction stalls the queue identically, so semantics (including
    the pacing the hardware workarounds rely on) are preserved exactly.
    """
    counter = [0]
    for f in nc.m.functions:
        for blk in f.blocks:
            insts = blk.instructions
            i = 0
            while i < len(insts):
                inst = insts[i]
                si = inst.sync_info
                tname = type(inst).__name__
                if (
                    si is not None
                    and len(si.on_wait) > 1
                    and not any(s in tname for s in _SKIP_TYPES)
                ):
                    waits = list(si.on_wait)
                    for w in waits[:-1]:
                        counter[0] += 1
                        nop = mybir.InstNoOp(
                            name=f"wsplit-{counter[0]}", ins=[], outs=[]
                        )
                        nop.engine = inst.engine
                        nop.sync_info = mybir.SyncInfo(on_wait=[w], on_update=[])
                        insts.insert(i, nop)
                        i += 1
                    inst.sync_info = mybir.SyncInfo(
                        on_wait=[waits[-1]], on_update=list(si.on_update)
                    )
                i += 1


_PROGRAM_CACHE = {}


def _get_program():
    if "nc" not in _PROGRAM_CACHE:
        _PROGRAM_CACHE["nc"] = _build_program()
    return _PROGRAM_CACHE["nc"]


def _transpose_to_pixel_major(x: np.ndarray) -> np.ndarray:
    """x fp32 [B, S, PX] -> bf16 [N_CORES, PX, TOKENS], cache-blocked."""
    xb = x.astype(BF16).reshape(N_CORES, TOKENS // P, P, PX)
    # per-block transpose: [core, blk, px, tok%128]; 32 KB blocks stay in L1
    xb = np.ascontiguousarray(xb.transpose(0, 1, 3, 2))
    # gather blocks per pixel row: inner runs stay 256 B contiguous
    xt = np.ascontiguousarray(xb.transpose(0, 2, 1, 3))
    return xt.reshape(N_CORES, PX, TOKENS)


def _make_in_maps(x: np.ndarray, kernel3x3: np.ndarray) -> list:
    x = np.asarray(x, dtype=np.float32)
    k3 = np.asarray(kernel3x3, dtype=np.float32)
    assert x.shape == (B, S, PX), x.shape
    assert k3.shape == (K, K), k3.shape
    m_bf = _build_conv_matrix(k3).astype(BF16)  # [128, 84]
    xt = _transpose_to_pixel_major(x)
    in_maps = []
    for i in range(N_CORES):
        x0m = np.concatenate([m_bf, xt[i, :, :CHUNK0]], axis=1)
        in_maps.append(
            {
                "x0m": np.ascontiguousarray(x0m),
                "xr": np.ascontiguousarray(xt[i, :, CHUNK0:]),
            }
        )
    return in_maps


def kernel(x: np.ndarray, kernel: np.ndarray) -> np.ndarray:
    nc = _get_program()
    in_maps = _make_in_maps(x, kernel)

    res = run_bass_kernel_spmd(nc, in_maps, list(range(N_CORES)))

    out = np.zeros((B, S, PX), dtype=np.float32)
    ov = out.reshape(N_CORES, TOKENS, PX)
    starts = [sum(CHUNK_SIZES[:i]) for i in range(len(CHUNK_SIZES))]
    for i in range(N_CORES):
        r = np.asarray(res.results[i]["out"])
        for c, csize in enumerate(CHUNK_SIZES):
            nblocks = csize // BLK
            col0 = (starts[c] // BLK) * OUT + c + 1  # skip the dep column
            # slab[p, b, o] = conv slot o of token starts[c] + b*128 + p
            slab = r[:, col0 : col0 + nblocks * OUT].reshape(P, nblocks, OUT)
            ov[i, starts[c] : starts[c] + csize, :OUT] = slab.transpose(
                1, 0, 2
            ).reshape(csize, OUT)
    return out
